# revision 23
# baseline (speedup 1.0000x reference)
"""Trainium2 Bass kernel for CombinedLoss (mse + bone_mse + hole_mse).

loss = mean(diff^2) + mean((bone*diff)^2) + mean((hole_dil*diff)^2)
with diff = y_pred - y_true, binary masks, and hole_dil a 15^3 binary box
dilation of hole0 = (y_true>=0.5)&(x<0.5).

Strategy: data-parallel over the D axis across 8 NeuronCores with an
8-left / 8-right slice halo (host zero-padded). All cores run an identical
SPMD program; per-partition partial sums are summed on the host.

Since masks are binary and bone = x1 | y1 = x1 + h0 (disjoint), the loss
collapses to sum(diff^2 * (1 + bone + hole)) / N. With x0 = (x < 0.5):
bone = 1 - x0 + h0, so the weight is 2 + v + hole with v = h0 - x0, and
the kernel keeps two accumulators: S1 = sum(sq), S2 = sum(sq*(v+hole));
the host combines 2*S1 + S2.

Dilation pipeline (counts are non-negative, only positivity survives the
single final threshold, and the three box-sums commute):
  - D pass: exact cumulative chain A[j] = A[j-1] + h0[j] (bf16 ints <= 48,
    exact), then T[d] = A[d+15] - A[d] batched per quad (big A tile).
  - W pass (BEFORE H): 3-add tree builds S8[w] = sum T[w..w+7]; the
    15-window is completed inside the H matmul by accumulating TWO
    w-shifted copies (delta in {0,7}) into the same PSUM - the overlap
    at the seam double-counts, which is harmless for positivity.
  - H pass: TensorEngine matmul with banded ones matrices, 16 psum-
    accumulated matmuls per quad (2 b_m x 2 h x 2 b_k x 2 delta).
  - ONE threshold: ScalarE Sign reads PSUM directly -> hole in {0,1}.

Combine: diff = yp - yt (DVE), sq = Square(diff) accum S1 (ScalarE),
bh2 = v + hole (DVE), prod = sq*bh2 (DVE), Copy(prod) accum S2 (ScalarE).

All elementwise work stays on the DVE (2x bf16 mode): offloading adds to
the Pool engine measurably slows the DVE down (SBUF contention from the
Q7 software loops), so Pool only does tiny border memsets. The pipeline
is software-pipelined in three stages per quad so the in-order DVE queue
never blocks on ScalarE/PE results of the same quad.

Layout: SBUF tiles are [128 part, 4 (d in quad), 2 (j), 256 (w)] with
partition p holding H row-pair (2p, 2p+j) -- each partition's DMA row is
1 KiB contiguous DRAM. The H-band matmul matrices are permuted to match.
"""

import os
import sys

import numpy as np

sys.path.insert(0, "/opt/trn_rl_repo")

D_FULL, H, W = 256, 256, 256
NCORES = 8
SLAB = D_FULL // NCORES          # 32 own slices per core
HALO = 7
LPAD = 8                         # left halo padding (8 keeps pairs aligned)
HSLAB = SLAB + 2 * LPAD          # 48 haloed slices; own slice d = index d+8
WPAD = W + 2 * HALO              # 270 padded W extent for the add tree
NTOT = float(D_FULL * H * W)

LAST_EXEC_NS = None
LAST_RESULT = None

_NC_CACHE = {}


def _band_blocks() -> np.ndarray:
    """lhsT blocks for the H-axis banded box-sum matmul, [128, 4*128] f32.

    Interleaved-H layout: partition p of a k/m block b holds H row 2p+b.
    Block (b_k, b_m) at [:, 128*(2*b_k+b_m):...]:
      B[k', m'] = 1 iff |(2k'+b_k) - (2m'+b_m)| <= 7.
    """
    k = np.arange(128)[:, None]
    m = np.arange(128)[None, :]
    blocks = []
    for b_k in (0, 1):
        for b_m in (0, 1):
            blocks.append((np.abs((2 * k + b_k) - (2 * m + b_m)) <= HALO))
    import ml_dtypes
    return np.concatenate(blocks, axis=1).astype(ml_dtypes.bfloat16)


def _build_nc():
    import concourse.bacc as bacc
    import concourse.mybir as mybir
    from concourse.tile import TileContext

    fp32 = mybir.dt.float32
    bf16 = mybir.dt.bfloat16
    OP = mybir.AluOpType
    ACT = mybir.ActivationFunctionType

    nc = bacc.Bacc(None, target_bir_lowering=False, debug=False)
    yp_d = nc.declare_dram_parameter("yp", [SLAB, H, W], bf16, isOutput=False)
    yt_d = nc.declare_dram_parameter("yt", [HSLAB, H, W], bf16, isOutput=False)
    xx_d = nc.declare_dram_parameter("xx", [HSLAB, H, W], bf16, isOutput=False)
    bd_d = nc.declare_dram_parameter("band", [128, 512], bf16, isOutput=False)
    hb_d = nc.declare_dram_parameter("hbias", [128, 1], fp32, isOutput=False)
    out_d = nc.declare_dram_parameter("out", [128, SLAB // 2], fp32, isOutput=True)

    NB = 4  # slices per quad tile

    def dram_quad(t, i):
        # slices [i, i+NB) -> [128 part, NB (d), 2 (j), 256 (w)], partition p
        # holds H rows (2p, 2p+1): per-partition run = 512 bf16 = 1 KiB
        return t[i:i + NB].rearrange("d (p j) w -> p d j w", p=128)

    NQH = HSLAB // NB   # 12 haloed quads
    NQ = SLAB // NB     # 8 own quads

    with TileContext(nc) as tc:
        with (
            tc.tile_pool(name="pconst", bufs=1) as pconst,
            tc.tile_pool(name="pin", bufs=1) as pin,
            tc.tile_pool(name="pmask", bufs=1) as pmask,
            tc.tile_pool(name="pdil", bufs=1) as pdil,
            tc.tile_pool(name="pcomb", bufs=1) as pcomb,
            tc.tile_pool(name="pps", bufs=2, space="PSUM") as pps,
        ):
            band_b = pconst.tile([128, 512], bf16, tag="band_b")
            nc.sync.dma_start(out=band_b[:, :], in_=bd_d[:, :])
            hbias_b = pconst.tile([128, 1], fp32, tag="hbias_b")
            nc.sync.dma_start(out=hbias_b[:, :], in_=hb_d[:, :])

            def bblk(b_k, b_m):
                o = 128 * (2 * b_k + b_m)
                return band_b[:, o:o + 128]

            # per-own-quad partial sums: slot 2q = sum(sq), 2q+1 = sum(sq*w2)
            acc = pconst.tile([128, 2 * NQ], fp32, tag="acc")

            yt_t = {}    # haloed quad index -> tile
            h0_t = {}
            v_t = {}     # own quad index -> h0 - x0

            # persistent cumulative sums A[j] = sum_{i<=j} h0[i], big tile so
            # T[d] = A[d+15] - A[d] batches at quad granularity
            A = pconst.tile([128, HSLAB, 2, W], bf16, tag="A")

            def S(quads, j):
                # slice view of a quad-tile dict, [128, 2, 256]
                return quads[j - j % NB][:, j % NB, :, :]

            def load_masks(jq):
                j0 = jq * NB
                yt = pin.tile([128, NB, 2, W], bf16, tag="yt", bufs=5)
                nc.sync.dma_start(out=yt[:, :, :, :], in_=dram_quad(yt_d, j0))
                xv = pin.tile([128, NB, 2, W], bf16, tag="xv", bufs=3)
                nc.sync.dma_start(out=xv[:, :, :, :], in_=dram_quad(xx_d, j0))
                yt_t[jq] = yt
                # slice j = HSLAB-1 is never read (chain stops at A[46]):
                # trim the last quad's mask ops to 3 slices
                nn = NB if j0 + NB < HSLAB else NB - 1
                # y1 = (yt >= 0.5) on the SCALAR engine: saturated sigmoid
                # sigmoid(1e9*yt - 0.49975e9) is exactly 0/1 (exp underflow /
                # bf16 round-to-1), splitting mask work off the busy DVE
                y1 = pmask.tile([128, NB, 2, W], bf16, tag="y1", bufs=1)
                nc.scalar.activation(y1[:, 0:nn, :, :], yt[:, 0:nn, :, :],
                                     ACT.Sigmoid, bias=hbias_b[:, 0:1], scale=1e9)
                x0 = pmask.tile([128, NB, 2, W], bf16, tag="x0", bufs=1)
                nc.vector.tensor_scalar(x0[:, 0:nn, :, :], xv[:, 0:nn, :, :], 0.5, None, OP.is_lt)
                h0 = pmask.tile([128, NB, 2, W], bf16, tag="h0", bufs=2, name=f"h0{jq}")
                nc.vector.tensor_tensor(h0[:, 0:nn, :, :], y1[:, 0:nn, :, :], x0[:, 0:nn, :, :], OP.mult)
                h0_t[j0] = h0
                # v = h0 - x0 on own slices (weight term bone+hole-1)
                q = jq - 2  # own quad index (haloed offset 8 = 2 quads)
                if 0 <= q < NQ:
                    v = pmask.tile([128, NB, 2, W], bf16, tag="v", bufs=6)
                    nc.vector.tensor_tensor(v[:, :, :, :], h0[:, :, :, :], x0[:, :, :, :], OP.subtract)
                    v_t[q] = v
                # cumulative chain A[j] = A[j-1] + h0[j]
                for j in range(j0, j0 + NB):
                    if j == 0:
                        nc.vector.tensor_copy(out=A[:, 0, :, :], in_=S(h0_t, 0))
                    elif j < HSLAB - 1:
                        nc.vector.tensor_tensor(A[:, j, :, :], A[:, j - 1, :, :], S(h0_t, j), OP.add)

            s1_t = {}
            yp_t = {}
            hole_t = {}
            sq_t = {}

            def dilate_stage1(q):
                # own quad q: T[d] = A[d+15] - A[d] (own d -> haloed j =
                # d+8, window [j-7, j+7] = haloed [d+1, d+15]), then the
                # widest W-tree add; issued a stage ahead of stage2.
                d0 = NB * q
                Tp = pdil.tile([128, NB, 2, WPAD], bf16, tag="Tp", bufs=2)
                nc.gpsimd.memset(Tp[:, :, :, 0:HALO], 0.0)
                nc.gpsimd.memset(Tp[:, :, :, W + HALO:WPAD], 0.0)
                nc.vector.tensor_tensor(
                    Tp[:, :, :, HALO:W + HALO],
                    A[:, d0 + 15:d0 + 19, :, :], A[:, d0:d0 + NB, :, :],
                    OP.subtract)
                s1 = pdil.tile([128, NB, 2, WPAD], bf16, tag="s1", bufs=2)
                nc.vector.tensor_tensor(s1[:, :, :, 0:269], Tp[:, :, :, 0:269], Tp[:, :, :, 1:270], OP.add)
                s1_t[q] = s1
                yp = pin.tile([128, NB, 2, W], bf16, tag="yp", bufs=2)
                nc.sync.dma_start(out=yp[:, :, :, :], in_=dram_quad(yp_d, d0))
                yp_t[q] = yp

            def dilate_stage2(q):
                d0 = NB * q
                # W pass: S8[w] = sum T-pad[w..w+7] via 3 adds; the 15-window
                # is completed by two shifted psum-accumulated matmuls
                s1 = s1_t.pop(q)
                s2 = pdil.tile([128, NB, 2, WPAD], bf16, tag="s2", bufs=2)
                nc.vector.tensor_tensor(s2[:, :, :, 0:267], s1[:, :, :, 0:267], s1[:, :, :, 2:269], OP.add)
                s8 = pdil.tile([128, NB, 2, WPAD], bf16, tag="s8", bufs=2)
                nc.vector.tensor_tensor(s8[:, :, :, 0:263], s2[:, :, :, 0:263], s2[:, :, :, 4:267], OP.add)
                # H pass + W completion: psum[h_out, u] =
                #   sum_{b_k, delta in {0,7}} band * S8[.., u+delta]
                ps = pps.tile([128, 2, NB, W], fp32, tag="ps")
                for b_m in (0, 1):
                    for h in (0, 1):
                        for b_k in (0, 1):
                            for i_d, dl in enumerate((0, HALO)):
                                nc.tensor.matmul(
                                    ps[:, b_m, 2 * h:2 * h + 2, :],
                                    bblk(b_k, b_m),
                                    s8[:, 2 * h:2 * h + 2, b_k, dl:dl + W],
                                    start=(b_k == 0 and i_d == 0),
                                    stop=(b_k == 1 and i_d == 1))
                # ONE threshold: counts > 0 -> {0,1}
                hole = pcomb.tile([128, NB, 2, W], bf16, tag="hole", bufs=2)
                nc.scalar.activation(
                    hole[:, :, :, :],
                    ps[:, :, :, :].rearrange("p b d w -> p d b w"),
                    ACT.Sign)
                hole_t[q] = hole
                # diff + sq have no dependency on the dilation: keep the
                # Scalar engine busy while Sign[q] completes
                yp = yp_t.pop(q)
                diff = pcomb.tile([128, NB, 2, W], bf16, tag="diff", bufs=2)
                nc.vector.tensor_tensor(diff[:, :, :, :], yp[:, :, :, :], yt_t[q + 2][:, :, :, :], OP.subtract)
                sq = pcomb.tile([128, NB, 2, W], bf16, tag="sq", bufs=2)
                nc.scalar.activation(sq[:, :, :, :], diff[:, :, :, :], ACT.Square,
                                     accum_out=acc[:, 2 * q:2 * q + 1])
                sq_t[q] = sq

            def combine_stage3(q):
                # weight w2 = bone + hole - 1 + 1 = (h0 - x0) + hole + 1;
                # loss uses 1 + bone + hole = 2 + v + hole, and
                # sum(sq*(2+v+hole)) = 2*sum(sq) + sum(sq*(v+hole)) -- fold
                # the constant on the host: acc slots hold sum(sq) and
                # sum(sq*(v+hole)).
                hole = hole_t.pop(q)
                sq = sq_t.pop(q)
                bh2 = pcomb.tile([128, NB, 2, W], bf16, tag="bh2", bufs=2)
                nc.vector.tensor_tensor(bh2[:, :, :, :], v_t.pop(q)[:, :, :, :], hole[:, :, :, :], OP.add)
                prod = pcomb.tile([128, NB, 2, W], bf16, tag="prod", bufs=2)
                nc.vector.tensor_tensor(prod[:, :, :, :], sq[:, :, :, :], bh2[:, :, :, :], OP.mult)
                scr = pcomb.tile([128, NB, 2, W], bf16, tag="scr", bufs=1)
                nc.scalar.activation(scr[:, :, :, :], prod[:, :, :, :], ACT.Copy,
                                     accum_out=acc[:, 2 * q + 1:2 * q + 2])

            for jq in range(NQH):
                if jq >= 5:
                    dilate_stage1(jq - 5)
                load_masks(jq)
                if jq >= 6:
                    dilate_stage2(jq - 6)
                if jq >= 7:
                    combine_stage3(jq - 7)
            dilate_stage1(NQ - 1)
            dilate_stage2(NQ - 2)
            combine_stage3(NQ - 3)
            dilate_stage2(NQ - 1)
            combine_stage3(NQ - 2)
            combine_stage3(NQ - 1)

            nc.sync.dma_start(out=out_d[:, :], in_=acc[:, :])

    nc.finalize()
    return nc


def _get_nc():
    if "nc" not in _NC_CACHE:
        _NC_CACHE["nc"] = _build_nc()
    return _NC_CACHE["nc"]


def _install_profile_bridge():
    """Register the axon NTFF profile hook that the image's antenv lacks,
    and stub out the S3 artifact upload (no creds in this container)."""
    import types

    import concourse.bass_utils as bu

    if "antenv.axon_hooks" not in sys.modules:
        try:
            from trn_agent_boot.trn_boot import _ntff_profile_via_ctypes

            hook = _ntff_profile_via_ctypes("/opt/axon/libaxon_pjrt.so")
            mod = types.ModuleType("antenv.axon_hooks")
            mod.get_axon_ntff_profile_hook = lambda: hook
            mod.set_axon_ntff_profile_hook = lambda h: None
            sys.modules["antenv.axon_hooks"] = mod
            import antenv

            antenv.axon_hooks = mod
        except Exception as e:  # degrade to trace-less run
            print(f"profile bridge unavailable: {e}", file=sys.stderr)
    bu.upload_artifacts = lambda tmpdir: tmpdir


def kernel(y_pred, y_true, x):
    global LAST_EXEC_NS, LAST_RESULT
    import ml_dtypes

    bf = ml_dtypes.bfloat16
    yp = np.asarray(y_pred, dtype=np.float32).reshape(D_FULL, H, W).astype(bf)
    yt = np.asarray(y_true, dtype=np.float32).reshape(D_FULL, H, W).astype(bf)
    xv = np.asarray(x, dtype=np.float32).reshape(D_FULL, H, W).astype(bf)

    band = _band_blocks()
    # sigmoid threshold bias: -1e9*theta with theta strictly between the
    # largest bf16 < 0.5 (0.498046875) and 0.5
    hbias = np.full((128, 1), np.float32(-0.49975) * np.float32(1e9), np.float32)
    in_maps = []
    for c in range(NCORES):
        g0 = c * SLAB - LPAD
        yt_s = np.zeros((HSLAB, H, W), bf)
        xx_s = np.zeros((HSLAB, H, W), bf)
        lo, hi = max(0, g0), min(D_FULL, g0 + HSLAB)
        yt_s[lo - g0:hi - g0] = yt[lo:hi]
        xx_s[lo - g0:hi - g0] = xv[lo:hi]
        in_maps.append({
            "yp": np.ascontiguousarray(yp[c * SLAB:(c + 1) * SLAB]),
            "yt": yt_s,
            "xx": xx_s,
            "band": band,
            "hbias": hbias,
        })

    from concourse.bass_utils import run_bass_kernel_spmd

    nc = _get_nc()
    trace = os.environ.get("KERNEL_TRACE", "0") == "1"
    if trace:
        _install_profile_bridge()
    res = run_bass_kernel_spmd(nc, in_maps, list(range(NCORES)), trace=trace)
    LAST_EXEC_NS = res.exec_time_ns
    LAST_RESULT = res

    tot = 0.0
    for r in res.results:
        o = np.asarray(r["out"], dtype=np.float64).reshape(128, SLAB // 4, 2)
        # slot 2q = sum(sq), 2q+1 = sum(sq*(v+hole)); weight = 2 + v + hole
        tot += 2.0 * o[:, :, 0].sum() + o[:, :, 1].sum()
    return np.asarray(tot / NTOT, dtype=np.float32)



# revision 24
# speedup vs baseline: 1.1824x; 1.1824x over previous
"""Trainium2 Bass kernel for CombinedLoss (mse + bone_mse + hole_mse).

loss = mean(diff^2) + mean((bone*diff)^2) + mean((hole_dil*diff)^2)
with diff = y_pred - y_true, binary masks, and hole_dil a 15^3 binary box
dilation of hole0 = (y_true>=0.5)&(x<0.5).

Strategy: data-parallel over the D axis across 8 NeuronCores with an
8-left / 8-right slice halo (host zero-padded). All cores run an identical
SPMD program; per-partition partial sums are summed on the host.

Since masks are binary and bone = x1 | y1 = x1 + h0 (disjoint), the loss
collapses to sum(diff^2 * (1 + bone + hole)) / N. With x0 = (x < 0.5):
bone = 1 - x0 + h0, so the weight is 2 + v + hole with v = h0 - x0, and
the kernel keeps two accumulators: S1 = sum(sq), S2 = sum(sq*(v+hole));
the host combines 2*S1 + S2.

Dilation pipeline (counts are non-negative, only positivity survives the
single final threshold, and the three box-sums commute):
  - D pass: exact cumulative chain A[j] = A[j-1] + h0[j] (bf16 ints <= 48,
    exact), then T[d] = A[d+15] - A[d] batched per quad (big A tile).
  - W pass (BEFORE H): 3-add tree builds S8[w] = sum T[w..w+7]; the
    15-window is completed inside the H matmul by accumulating TWO
    w-shifted copies (delta in {0,7}) into the same PSUM - the overlap
    at the seam double-counts, which is harmless for positivity.
  - H pass: TensorEngine matmul with banded ones matrices, 16 psum-
    accumulated matmuls per quad (2 b_m x 2 h x 2 b_k x 2 delta).
  - ONE threshold: ScalarE Sign reads PSUM directly -> hole in {0,1}.

Combine: diff = yp - yt (DVE), sq = Square(diff) accum S1 (ScalarE),
bh2 = v + hole (DVE), prod = sq*bh2 (DVE), Copy(prod) accum S2 (ScalarE).

All elementwise work stays on the DVE (2x bf16 mode): offloading adds to
the Pool engine measurably slows the DVE down (SBUF contention from the
Q7 software loops), so Pool only does tiny border memsets. The pipeline
is software-pipelined in three stages per quad so the in-order DVE queue
never blocks on ScalarE/PE results of the same quad.

Layout: SBUF tiles are [128 part, 4 (d in quad), 2 (j), 256 (w)] with
partition p holding H row-pair (2p, 2p+j) -- each partition's DMA row is
1 KiB contiguous DRAM. The H-band matmul matrices are permuted to match.
"""

import os
import sys

import numpy as np

sys.path.insert(0, "/opt/trn_rl_repo")

D_FULL, H, W = 256, 256, 256
NCORES = 8
SLAB = D_FULL // NCORES          # 32 own slices per core
HALO = 7
LPAD = 8                         # left halo padding (8 keeps pairs aligned)
HSLAB = SLAB + 2 * LPAD          # 48 haloed slices; own slice d = index d+8
WPAD = W + 2 * HALO              # 270 padded W extent for the add tree
NTOT = float(D_FULL * H * W)

LAST_EXEC_NS = None
LAST_RESULT = None

_NC_CACHE = {}


def _band_blocks() -> np.ndarray:
    """lhsT blocks for the H-axis banded box-sum matmul, [128, 4*128] f32.

    Interleaved-H layout: partition p of a k/m block b holds H row 2p+b.
    Block (b_k, b_m) at [:, 128*(2*b_k+b_m):...]:
      B[k', m'] = 1 iff |(2k'+b_k) - (2m'+b_m)| <= 7.
    """
    k = np.arange(128)[:, None]
    m = np.arange(128)[None, :]
    blocks = []
    for b_k in (0, 1):
        for b_m in (0, 1):
            blocks.append((np.abs((2 * k + b_k) - (2 * m + b_m)) <= HALO))
    import ml_dtypes
    return np.concatenate(blocks, axis=1).astype(ml_dtypes.bfloat16)


def _build_nc():
    import concourse.bacc as bacc
    import concourse.mybir as mybir
    from concourse.tile import TileContext

    fp32 = mybir.dt.float32
    bf16 = mybir.dt.bfloat16
    OP = mybir.AluOpType
    ACT = mybir.ActivationFunctionType

    nc = bacc.Bacc(None, target_bir_lowering=False, debug=False)
    yp_d = nc.declare_dram_parameter("yp", [SLAB, H, W], bf16, isOutput=False)
    yt_d = nc.declare_dram_parameter("yt", [HSLAB, H, W], bf16, isOutput=False)
    xx_d = nc.declare_dram_parameter("xx", [HSLAB, H, W], bf16, isOutput=False)
    bd_d = nc.declare_dram_parameter("band", [128, 512], bf16, isOutput=False)
    hb_d = nc.declare_dram_parameter("hbias", [128, 1], fp32, isOutput=False)
    out_d = nc.declare_dram_parameter("out", [128, SLAB // 2], fp32, isOutput=True)

    NB = 4  # slices per quad tile

    def dram_quad(t, i):
        # slices [i, i+NB) -> [128 part, NB (d), 2 (j), 256 (w)], partition p
        # holds H rows (2p, 2p+1): per-partition run = 512 bf16 = 1 KiB
        return t[i:i + NB].rearrange("d (p j) w -> p d j w", p=128)

    NQH = HSLAB // NB   # 12 haloed quads
    NQ = SLAB // NB     # 8 own quads

    with TileContext(nc) as tc:
        with (
            tc.tile_pool(name="pconst", bufs=1) as pconst,
            tc.tile_pool(name="pin", bufs=1) as pin,
            tc.tile_pool(name="pmask", bufs=1) as pmask,
            tc.tile_pool(name="pdil", bufs=1) as pdil,
            tc.tile_pool(name="pcomb", bufs=1) as pcomb,
            tc.tile_pool(name="pps", bufs=2, space="PSUM") as pps,
        ):
            band_b = pconst.tile([128, 512], bf16, tag="band_b")
            nc.sync.dma_start(out=band_b[:, :], in_=bd_d[:, :])
            hbias_b = pconst.tile([128, 1], fp32, tag="hbias_b")
            nc.sync.dma_start(out=hbias_b[:, :], in_=hb_d[:, :])

            def bblk(b_k, b_m):
                o = 128 * (2 * b_k + b_m)
                return band_b[:, o:o + 128]

            # per-own-quad partial sums: slot 2q = sum(sq), 2q+1 = sum(sq*w2)
            acc = pconst.tile([128, 2 * NQ], fp32, tag="acc")

            yt_t = {}    # haloed quad index -> tile
            h0_t = {}
            v_t = {}     # own quad index -> h0 - x0

            # persistent cumulative sums A[j] = sum_{i<=j} h0[i], big tile so
            # T[d] = A[d+15] - A[d] batches at quad granularity
            A = pconst.tile([128, HSLAB, 2, W], bf16, tag="A")

            def S(quads, j):
                # slice view of a quad-tile dict, [128, 2, 256]
                return quads[j - j % NB][:, j % NB, :, :]

            def load_masks(jq):
                j0 = jq * NB
                yt = pin.tile([128, NB, 2, W], bf16, tag="yt", bufs=5)
                nc.sync.dma_start(out=yt[:, :, :, :], in_=dram_quad(yt_d, j0))
                xv = pin.tile([128, NB, 2, W], bf16, tag="xv", bufs=2)
                nc.sync.dma_start(out=xv[:, :, :, :], in_=dram_quad(xx_d, j0))
                yt_t[jq] = yt
                # slice j = HSLAB-1 is never read (chain stops at A[46]):
                # trim the last quad's mask ops to 3 slices
                nn = NB if j0 + NB < HSLAB else NB - 1
                # y1 = (yt >= 0.5) on the SCALAR engine: saturated sigmoid
                # sigmoid(1e9*yt - 0.49975e9) is exactly 0/1 (exp underflow /
                # bf16 round-to-1), splitting mask work off the busy DVE
                y1 = pmask.tile([128, NB, 2, W], bf16, tag="y1", bufs=1)
                nc.scalar.activation(y1[:, 0:nn, :, :], yt[:, 0:nn, :, :],
                                     ACT.Sigmoid, bias=hbias_b[:, 0:1], scale=1e9)
                x0 = pmask.tile([128, NB, 2, W], bf16, tag="x0", bufs=1)
                nc.vector.tensor_scalar(x0[:, 0:nn, :, :], xv[:, 0:nn, :, :], 0.5, None, OP.is_lt)
                h0 = pmask.tile([128, NB, 2, W], bf16, tag="h0", bufs=2, name=f"h0{jq}")
                nc.vector.tensor_tensor(h0[:, 0:nn, :, :], y1[:, 0:nn, :, :], x0[:, 0:nn, :, :], OP.mult)
                h0_t[j0] = h0
                # v = h0 - x0 on own slices (weight term bone+hole-1)
                q = jq - 2  # own quad index (haloed offset 8 = 2 quads)
                if 0 <= q < NQ:
                    v = pmask.tile([128, NB, 2, W], bf16, tag="v", bufs=6)
                    nc.vector.tensor_tensor(v[:, :, :, :], h0[:, :, :, :], x0[:, :, :, :], OP.subtract)
                    v_t[q] = v
                # cumulative chain A[j] = A[j-1] + h0[j]
                for j in range(j0, j0 + NB):
                    if j == 0:
                        nc.vector.tensor_copy(out=A[:, 0, :, :], in_=S(h0_t, 0))
                    elif j < HSLAB - 1:
                        nc.vector.tensor_tensor(A[:, j, :, :], A[:, j - 1, :, :], S(h0_t, j), OP.add)

            s1_t = {}
            yp_t = {}
            hole_t = {}
            sq_t = {}

            def dilate_stage1(q):
                # own quad q: T[d] = A[d+15] - A[d] (own d -> haloed j =
                # d+8, window [j-7, j+7] = haloed [d+1, d+15]), then the
                # widest W-tree add; issued a stage ahead of stage2.
                d0 = NB * q
                Tp = pdil.tile([128, NB, 2, WPAD], bf16, tag="Tp", bufs=2)
                nc.gpsimd.memset(Tp[:, :, :, 0:HALO], 0.0)
                nc.gpsimd.memset(Tp[:, :, :, W + HALO:WPAD], 0.0)
                nc.vector.tensor_tensor(
                    Tp[:, :, :, HALO:W + HALO],
                    A[:, d0 + 15:d0 + 19, :, :], A[:, d0:d0 + NB, :, :],
                    OP.subtract)
                s1 = pdil.tile([128, NB, 2, WPAD], bf16, tag="s1", bufs=2)
                nc.vector.tensor_tensor(s1[:, :, :, 0:269], Tp[:, :, :, 0:269], Tp[:, :, :, 1:270], OP.add)
                s1_t[q] = s1
                yp = pin.tile([128, NB, 2, W], bf16, tag="yp", bufs=2)
                nc.sync.dma_start(out=yp[:, :, :, :], in_=dram_quad(yp_d, d0))
                yp_t[q] = yp

            def dilate_stage2(q):
                d0 = NB * q
                # W pass: S8[w] = sum T-pad[w..w+7] via 3 adds; the 15-window
                # is completed by two shifted psum-accumulated matmuls
                s1 = s1_t.pop(q)
                s2 = pdil.tile([128, NB, 2, WPAD], bf16, tag="s2", bufs=2)
                nc.vector.tensor_tensor(s2[:, :, :, 0:267], s1[:, :, :, 0:267], s1[:, :, :, 2:269], OP.add)
                s8 = pdil.tile([128, NB, 2, WPAD], bf16, tag="s8", bufs=2)
                nc.vector.tensor_tensor(s8[:, :, :, 0:263], s2[:, :, :, 0:263], s2[:, :, :, 4:267], OP.add)
                # H pass + W completion: psum[h_out, u] =
                #   sum_{b_k, delta in {0,7}} band * S8[.., u+delta]
                ps = pps.tile([128, 2, NB, W], fp32, tag="ps")
                for b_m in (0, 1):
                    for h in (0, 1):
                        for b_k in (0, 1):
                            for i_d, dl in enumerate((0, HALO)):
                                nc.tensor.matmul(
                                    ps[:, b_m, 2 * h:2 * h + 2, :],
                                    bblk(b_k, b_m),
                                    s8[:, 2 * h:2 * h + 2, b_k, dl:dl + W],
                                    start=(b_k == 0 and i_d == 0),
                                    stop=(b_k == 1 and i_d == 1))
                # ONE threshold: counts > 0 -> {0,1}
                hole = pcomb.tile([128, NB, 2, W], bf16, tag="hole", bufs=2)
                nc.scalar.activation(
                    hole[:, :, :, :],
                    ps[:, :, :, :].rearrange("p b d w -> p d b w"),
                    ACT.Sign)
                hole_t[q] = hole
                # diff + sq have no dependency on the dilation: keep the
                # Scalar engine busy while Sign[q] completes
                yp = yp_t.pop(q)
                diff = pcomb.tile([128, NB, 2, W], bf16, tag="diff", bufs=2)
                nc.vector.tensor_tensor(diff[:, :, :, :], yp[:, :, :, :], yt_t[q + 2][:, :, :, :], OP.subtract)
                sq = pcomb.tile([128, NB, 2, W], bf16, tag="sq", bufs=2)
                nc.scalar.activation(sq[:, :, :, :], diff[:, :, :, :], ACT.Square,
                                     accum_out=acc[:, 2 * q:2 * q + 1])
                sq_t[q] = sq

            def combine_stage3(q):
                # weight w2 = bone + hole - 1 + 1 = (h0 - x0) + hole + 1;
                # loss uses 1 + bone + hole = 2 + v + hole, and
                # sum(sq*(2+v+hole)) = 2*sum(sq) + sum(sq*(v+hole)) -- fold
                # the constant on the host: acc slots hold sum(sq) and
                # sum(sq*(v+hole)).
                hole = hole_t.pop(q)
                sq = sq_t.pop(q)
                bh2 = pcomb.tile([128, NB, 2, W], bf16, tag="bh2", bufs=2)
                nc.vector.tensor_tensor(bh2[:, :, :, :], v_t.pop(q)[:, :, :, :], hole[:, :, :, :], OP.add)
                prod = pcomb.tile([128, NB, 2, W], bf16, tag="prod", bufs=2)
                nc.vector.tensor_tensor(prod[:, :, :, :], sq[:, :, :, :], bh2[:, :, :, :], OP.mult)
                scr = pcomb.tile([128, NB, 2, W], bf16, tag="scr", bufs=1)
                nc.scalar.activation(scr[:, :, :, :], prod[:, :, :, :], ACT.Copy,
                                     accum_out=acc[:, 2 * q + 1:2 * q + 2])

            for jq in range(NQH):
                if jq >= 5:
                    dilate_stage1(jq - 5)
                load_masks(jq)
                if jq >= 6:
                    dilate_stage2(jq - 6)
                if jq >= 7:
                    combine_stage3(jq - 7)
            dilate_stage1(NQ - 1)
            dilate_stage2(NQ - 2)
            combine_stage3(NQ - 3)
            dilate_stage2(NQ - 1)
            combine_stage3(NQ - 2)
            combine_stage3(NQ - 1)

            nc.sync.dma_start(out=out_d[:, :], in_=acc[:, :])

    nc.finalize()
    return nc


def _get_nc():
    if "nc" not in _NC_CACHE:
        _NC_CACHE["nc"] = _build_nc()
    return _NC_CACHE["nc"]


def _install_profile_bridge():
    """Register the axon NTFF profile hook that the image's antenv lacks,
    and stub out the S3 artifact upload (no creds in this container)."""
    import types

    import concourse.bass_utils as bu

    if "antenv.axon_hooks" not in sys.modules:
        try:
            from trn_agent_boot.trn_boot import _ntff_profile_via_ctypes

            hook = _ntff_profile_via_ctypes("/opt/axon/libaxon_pjrt.so")
            mod = types.ModuleType("antenv.axon_hooks")
            mod.get_axon_ntff_profile_hook = lambda: hook
            mod.set_axon_ntff_profile_hook = lambda h: None
            sys.modules["antenv.axon_hooks"] = mod
            import antenv

            antenv.axon_hooks = mod
        except Exception as e:  # degrade to trace-less run
            print(f"profile bridge unavailable: {e}", file=sys.stderr)
    bu.upload_artifacts = lambda tmpdir: tmpdir


def kernel(y_pred, y_true, x):
    global LAST_EXEC_NS, LAST_RESULT
    import ml_dtypes

    bf = ml_dtypes.bfloat16
    yp = np.asarray(y_pred, dtype=np.float32).reshape(D_FULL, H, W).astype(bf)
    yt = np.asarray(y_true, dtype=np.float32).reshape(D_FULL, H, W).astype(bf)
    xv = np.asarray(x, dtype=np.float32).reshape(D_FULL, H, W).astype(bf)

    band = _band_blocks()
    # sigmoid threshold bias: -1e9*theta with theta strictly between the
    # largest bf16 < 0.5 (0.498046875) and 0.5
    hbias = np.full((128, 1), np.float32(-0.49975) * np.float32(1e9), np.float32)
    in_maps = []
    for c in range(NCORES):
        g0 = c * SLAB - LPAD
        yt_s = np.zeros((HSLAB, H, W), bf)
        xx_s = np.zeros((HSLAB, H, W), bf)
        lo, hi = max(0, g0), min(D_FULL, g0 + HSLAB)
        yt_s[lo - g0:hi - g0] = yt[lo:hi]
        xx_s[lo - g0:hi - g0] = xv[lo:hi]
        in_maps.append({
            "yp": np.ascontiguousarray(yp[c * SLAB:(c + 1) * SLAB]),
            "yt": yt_s,
            "xx": xx_s,
            "band": band,
            "hbias": hbias,
        })

    from concourse.bass_utils import run_bass_kernel_spmd

    nc = _get_nc()
    trace = os.environ.get("KERNEL_TRACE", "0") == "1"
    if trace:
        _install_profile_bridge()
    res = run_bass_kernel_spmd(nc, in_maps, list(range(NCORES)), trace=trace)
    LAST_EXEC_NS = res.exec_time_ns
    LAST_RESULT = res

    tot = 0.0
    for r in res.results:
        o = np.asarray(r["out"], dtype=np.float64).reshape(128, SLAB // 4, 2)
        # slot 2q = sum(sq), 2q+1 = sum(sq*(v+hole)); weight = 2 + v + hole
        tot += 2.0 * o[:, :, 0].sum() + o[:, :, 1].sum()
    return np.asarray(tot / NTOT, dtype=np.float32)

